# revision 8
# baseline (speedup 1.0000x reference)
"""Trainium2 Bass kernel for a dense transformer decoder layer.

Shapes (hardcoded): B=2, S=2048, D=1024, H=16, HD=64, FF=4096, fp32 I/O.

Single fused launch, token-parallel over 8 cores (512 query tokens each;
batch b owned by cores 4b..4b+3). Each core redundantly computes K/V for
its batch's mask-compacted key tokens (host drops mask==0 keys and
zero-pads to a chunk multiple).

LN1 is folded into the QKV GEMMs algebraically so the preamble has no
normalize pass and no activation transposes: the host ships x^T (a pure
layout transform) in fp8 and the GEMMs run on raw x^T, with the per-token
mean handled as a rank-2 correction appended to each PSUM accumulation
chain (lhsT = [colsum(W); bias], rhs = [-mu; std] rows) and the 1/std
factor applied per-partition at the cheapest point downstream:
  - K: deferred all the way to the softmax exp (activation scale AP),
  - Q: one broadcast-tile multiply on the PSUM->SBUF copy,
  - V: folded into the existing rstd*oc per-partition copy multiply.
K's bias is dropped entirely (softmax-invariant). bo and b2 are applied
as rank-1 rows appended to the Wo / W2 PSUM chains instead of full-tile
adds. Token stats come from token-major fp8 copies via bn_stats.

Precision: fp8e4m3 with DoubleRow matmuls for QKV projections, scores
operands, exp weights, V, and Wo; LN statistics, residual stream and
softmax normalization fp32; FFN in bf16 (fp8 exceeds the error budget).
"""

import os
import numpy as np
import ml_dtypes
from contextlib import ExitStack

import concourse.bass as bass
from concourse import bacc
import concourse.mybir as mybir
import concourse.tile as tile
from concourse.bass_utils import run_bass_kernel_spmd
from concourse.masks import make_identity

B, S, D, H, FF = 2, 2048, 1024, 16, 4096
HD = D // H
EPS = 1e-5
NCORES = 8
TOK = (B * S) // NCORES          # 512 query tokens per core
P = 128
DC = D // P                      # 8 contraction chunks
SBLK = TOK // P                  # 4 s-blocks of 128
FB = FF // P                     # 32 ff blocks of 128

F32 = mybir.dt.float32
BF16 = mybir.dt.bfloat16
FP8 = mybir.dt.float8e4
AF = mybir.ActivationFunctionType
ALU = mybir.AluOpType
DR = mybir.MatmulPerfMode.DoubleRow

bf16_np = ml_dtypes.bfloat16
fp8_np = ml_dtypes.float8_e4m3

T_PAD = 1152  # compacted key columns (only mask==1 keys kept, zero-padded)


def _ln_tile(nc, pools, x_tile, out_tile, eps_sb):
    """LayerNorm (no affine) of one [128, D] fp32 tile into out_tile."""
    stats = pools.tile([P, 2, 6], F32, tag="ln_stats")
    mv = pools.tile([P, 2], F32, tag="ln_mv")
    xg = x_tile.rearrange("p (g d) -> p g d", g=2)
    for g in range(2):
        nc.vector.bn_stats(out=stats[:, g, :], in_=xg[:, g, :])
    nc.vector.bn_aggr(out=mv[:], in_=stats[:])
    mean = mv[:, 0:1]
    std = pools.tile([P, 1], F32, tag="ln_std")
    nc.scalar.activation(out=std, in_=mv[:, 1:2], func=AF.Sqrt, bias=eps_sb, scale=1.0)
    nc.vector.reciprocal(out=std, in_=std)
    nc.vector.tensor_scalar(
        out=out_tile,
        in0=x_tile,
        scalar1=mean,
        scalar2=std,
        op0=ALU.subtract,
        op1=ALU.mult,
    )


def _build_fused(TKEY=T_PAD):
    NCH = TKEY // P                  # real key chunks (9 for 1152)
    NCHP = NCH + (NCH % 2)           # padded to even for DoubleRow (10)
    nc = bacc.Bacc(None, target_bir_lowering=False, debug=False)
    x_d = nc.declare_dram_parameter("x", [TOK, D], F32, isOutput=False)
    xq_d = nc.declare_dram_parameter("xq", [TOK, D], FP8, isOutput=False)
    xk_d = nc.declare_dram_parameter("xk", [TKEY, D], FP8, isOutput=False)
    xt_d = nc.declare_dram_parameter("xt", [P, DC * TOK], FP8, isOutput=False)
    xkt_d = nc.declare_dram_parameter("xkt", [P, DC * TKEY], FP8, isOutput=False)
    oc_d = nc.declare_dram_parameter("oc", [P, NCH], F32, isOutput=False)
    wq_d = nc.declare_dram_parameter("wq", [P, DC * D], FP8, isOutput=False)
    wk_d = nc.declare_dram_parameter("wk", [P, DC * D], FP8, isOutput=False)
    wv_d = nc.declare_dram_parameter("wv", [P, DC * D], FP8, isOutput=False)
    csq_d = nc.declare_dram_parameter("csq", [2, D], BF16, isOutput=False)
    csk_d = nc.declare_dram_parameter("csk", [1, D], BF16, isOutput=False)
    csv_d = nc.declare_dram_parameter("csv", [2, D], BF16, isOutput=False)
    wo_d = nc.declare_dram_parameter("wo", [P, DC * D], FP8, isOutput=False)
    bor_d = nc.declare_dram_parameter("bor", [1, D], BF16, isOutput=False)
    b2r_d = nc.declare_dram_parameter("b2r", [1, D], BF16, isOutput=False)
    w1_d = nc.declare_dram_parameter("w1", [D, FF], BF16, isOutput=False)
    b1_d = nc.declare_dram_parameter("b1", [P, FB], F32, isOutput=False)
    w2_d = nc.declare_dram_parameter("w2", [P, FB * D], BF16, isOutput=False)
    out_d = nc.declare_dram_parameter("out", [TOK, D], F32, isOutput=True)

    with tile.TileContext(nc) as tc, ExitStack() as ctx:
        glob = ctx.enter_context(tc.tile_pool(name="glob", bufs=1))

        ident = glob.tile([P, P], BF16)
        make_identity(nc, ident)
        eps_sb = glob.tile([P, 1], F32)
        nc.vector.memset(eps_sb, EPS)
        ones_sb = glob.tile([1, 64], BF16)
        nc.vector.memset(ones_sb, 1.0)
        ones1b = glob.tile([1, P], BF16)
        nc.vector.memset(ones1b, 1.0)
        negone_sb = glob.tile([P, 1], F32)
        nc.vector.memset(negone_sb, -1.0)
        onesh_sb = glob.tile([P, H], F32)
        nc.vector.memset(onesh_sb, 1.0)

        qt_sb = glob.tile([P, DC, TOK], FP8)
        kt_sb = glob.tile([P, DC, TKEY], FP8)
        va_sb = glob.tile([P, NCHP, H, HD + 1], FP8)
        ctxu_sb = glob.tile([P, DC, TOK], BF16)      # unnormalized ctx^T
        ctxn_sb = glob.tile([P, DC, TOK], FP8)       # normalized ctx^T
        exp_bufs = [glob.tile([P, NCHP, 2, TOK], FP8, name=f"expb{i}")
                    for i in range(3)]
        if NCHP != NCH:
            for eb in exp_bufs:
                nc.gpsimd.memset(eb[:, NCH:NCHP, :, :], 0.0)
            nc.gpsimd.memset(va_sb[:, NCH:NCHP, :, :], 0.0)

        # ---- input DMAs; stats-feeding tiles go first on the sync queue
        oc_sb = glob.tile([P, NCH], F32)
        nc.sync.dma_start(out=oc_sb, in_=oc_d[:, :])
        bor_sb = glob.tile([1, D], BF16)
        b2r_sb = glob.tile([1, D], BF16)
        b1_sb = glob.tile([P, FB], F32)
        wo_sb = glob.tile([P, DC, D], FP8)
        resid_sb = glob.tile([P, SBLK, D], F32)
        z2t_sb = glob.tile([P, DC, TOK], BF16)
        x_tiles = []

        with tc.tile_pool(name="qkv", bufs=1) as qkvp, \
             tc.tile_pool(name="lnw", bufs=2) as lnw, \
             tc.tile_pool(name="attn_sc", bufs=3, space="PSUM") as sc_psum, \
             tc.tile_pool(name="attn_cx", bufs=2, space="PSUM") as cx_psum, \
             tc.tile_pool(name="attn_wk", bufs=1) as awork:
            xq_tiles = []
            for sb in range(SBLK):
                t = qkvp.tile([P, D], FP8, name=f"xq{sb}")
                nc.sync.dma_start(out=t, in_=xq_d[sb * P:(sb + 1) * P, :])
                xq_tiles.append(t)
            xk_tiles = []
            for sb in range(NCH):
                t = qkvp.tile([P, D], FP8, name=f"xk{sb}")
                nc.sync.dma_start(out=t, in_=xk_d[sb * P:(sb + 1) * P, :])
                xk_tiles.append(t)
            xt_sb = qkvp.tile([P, DC, TOK], FP8)
            nc.sync.dma_start(
                out=xt_sb, in_=xt_d[:].rearrange("p (c n) -> p c n", c=DC))
            xkt_sb = qkvp.tile([P, DC, TKEY], FP8)
            xkt_ap = xkt_d[:].rearrange("p (c n) -> p c n", c=DC)
            for c0 in range(0, TKEY, 512):
                cw = min(512, TKEY - c0)
                nc.sync.dma_start(out=xkt_sb[:, :, c0:c0 + cw],
                                  in_=xkt_ap[:, :, c0:c0 + cw])
            for sb in range(SBLK):
                xt_ = glob.tile([P, D], F32, name=f"x{sb}")
                nc.sync.dma_start(out=xt_, in_=x_d[sb * P:(sb + 1) * P, :])
                x_tiles.append(xt_)

            # weights on the gpsimd issue queue
            wq_sb = qkvp.tile([P, DC, D], FP8)
            wk_sb = qkvp.tile([P, DC, D], FP8)
            wv_sb = qkvp.tile([P, DC, D], FP8)
            for dc in range(DC):
                nc.gpsimd.dma_start(out=wk_sb[:, dc, :],
                                    in_=wk_d[:, dc * D:(dc + 1) * D])
                nc.gpsimd.dma_start(out=wq_sb[:, dc, :],
                                    in_=wq_d[:, dc * D:(dc + 1) * D])
            csq_sb = qkvp.tile([2, D], BF16)
            csk_sb = qkvp.tile([1, D], BF16)
            csv_sb = qkvp.tile([2, D], BF16)
            nc.gpsimd.dma_start(out=csk_sb, in_=csk_d[:, :])
            nc.gpsimd.dma_start(out=csq_sb, in_=csq_d[:, :])
            nc.gpsimd.dma_start(out=csv_sb, in_=csv_d[:, :])
            for dc in range(DC):
                nc.gpsimd.dma_start(out=wv_sb[:, dc, :],
                                    in_=wv_d[:, dc * D:(dc + 1) * D])
            nc.gpsimd.dma_start(
                out=wo_sb, in_=wo_d[:].rearrange("p (c n) -> p c n", c=DC))
            nc.gpsimd.dma_start(out=bor_sb, in_=bor_d[:, :])
            nc.gpsimd.dma_start(out=b2r_sb, in_=b2r_d[:, :])
            nc.gpsimd.dma_start(out=b1_sb, in_=b1_d[:, :])

            # ---- per-token LN1 stats (no normalize, no transposed z1)
            qrows = qkvp.tile([3, TOK], BF16)    # rows: -mu, std, rstd
            qrstd_row = qkvp.tile([1, TOK], BF16)
            krows = qkvp.tile([3, TKEY], BF16)
            rstdk_sb = qkvp.tile([P, NCH], F32)  # per-key rstd (exp scale)
            rstdoc_sb = qkvp.tile([P, NCH], F32)
            rstdB = qkvp.tile([P, TOK], BF16)    # broadcast rstd_q

            def stats_tile(src, rows_sb, col, key_idx=None):
                bstats = lnw.tile([P, 2, 6], F32, tag="bn")
                mv = lnw.tile([P, 2], F32, tag="mv")
                xg = src.rearrange("p (g d) -> p g d", g=2)
                nc.vector.bn_stats(out=bstats[:, 0, :], in_=xg[:, 0, :])
                nc.vector.bn_stats(out=bstats[:, 1, :], in_=xg[:, 1, :])
                nc.vector.bn_aggr(out=mv[:], in_=bstats[:])
                std = lnw.tile([P, 1], F32, tag="std")
                nc.scalar.activation(out=std, in_=mv[:, 1:2], func=AF.Sqrt,
                                     bias=eps_sb, scale=1.0)
                if key_idx is not None:
                    rstd = rstdk_sb[:, key_idx:key_idx + 1]
                else:
                    rstd = lnw.tile([P, 1], F32, tag="rstd")
                nc.vector.reciprocal(out=rstd, in_=std)
                s3 = lnw.tile([P, 3], BF16, tag="s3")
                nc.vector.tensor_scalar(out=s3[:, 0:1], in0=mv[:, 0:1],
                                        scalar1=negone_sb, scalar2=None,
                                        op0=ALU.mult)
                nc.vector.tensor_copy(out=s3[:, 1:2], in_=std)
                nc.vector.tensor_copy(out=s3[:, 2:3], in_=rstd)
                if key_idx is not None:
                    nc.vector.tensor_mul(
                        out=rstdoc_sb[:, key_idx:key_idx + 1], in0=rstd,
                        in1=oc_sb[:, key_idx:key_idx + 1])
                pt = sc_psum.tile([3, P], BF16, tag="sc")
                nc.tensor.transpose(pt, s3, ident)
                nc.vector.tensor_copy(out=rows_sb[:, col:col + P], in_=pt)
                if key_idx is None:
                    pt1 = sc_psum.tile([1, P], BF16, tag="sc")
                    nc.tensor.transpose(pt1, s3[:, 2:3], ident)
                    nc.vector.tensor_copy(out=qrstd_row[:, col:col + P],
                                          in_=pt1)

            def q_pair(pair):
                pq = cx_psum.tile([P, 512], F32, tag="cx")
                for i in range(DC // 2):
                    nc.tensor.matmul(
                        pq,
                        lhsT=wq_sb[:, 2 * i:2 * i + 2, pair * P:(pair + 1) * P],
                        rhs=xt_sb[:, 2 * i:2 * i + 2, :],
                        start=(i == 0), stop=False,
                        perf_mode=DR,
                    )
                nc.tensor.matmul(
                    pq, lhsT=csq_sb[:, pair * P:(pair + 1) * P],
                    rhs=qrows[0:2, :], start=False, stop=True)
                nc.vector.tensor_mul(out=qt_sb[:, pair, :], in0=pq, in1=rstdB)

            KCOPY = [nc.vector, nc.scalar, nc.vector]

            def k_span(pair, c0, eng_i):
                cw = min(512, TKEY - c0)
                pk = cx_psum.tile([P, 512], F32, tag="cx")
                for i in range(DC // 2):
                    nc.tensor.matmul(
                        pk[:, 0:cw],
                        lhsT=wk_sb[:, 2 * i:2 * i + 2, pair * P:(pair + 1) * P],
                        rhs=xkt_sb[:, 2 * i:2 * i + 2, c0:c0 + cw],
                        start=(i == 0), stop=False,
                        perf_mode=DR,
                    )
                nc.tensor.matmul(
                    pk[:, 0:cw], lhsT=csk_sb[:, pair * P:(pair + 1) * P],
                    rhs=krows[0:1, c0:c0 + cw], start=False, stop=True)
                eng = KCOPY[eng_i % 3]
                if eng is nc.scalar:
                    eng.copy(out=kt_sb[:, pair, c0:c0 + cw], in_=pk[:, 0:cw])
                else:
                    eng.tensor_copy(out=kt_sb[:, pair, c0:c0 + cw],
                                    in_=pk[:, 0:cw])

            def scores_tb(pair, tb):
                et = exp_bufs[pair % 3]
                ps = sc_psum.tile([P, 2, TOK], F32, tag="sc")
                for hi in range(2):
                    po = 64 * hi
                    nc.tensor.matmul(
                        ps[:, hi, :],
                        lhsT=kt_sb[po:po + 64, pair, tb * P:(tb + 1) * P],
                        rhs=qt_sb[po:po + 64, pair, :],
                        start=True, stop=True,
                        tile_position=(po, 0),
                    )
                nc.scalar.activation(
                    out=et[:, tb, :, :], in_=ps, func=AF.Exp,
                    bias=negone_sb, scale=rstdk_sb[:, tb:tb + 1])

            def v_build(sbs):
                for sb in sbs:
                    for vh in range(2):
                        pv = cx_psum.tile([P, 512], F32, tag="cx")
                        for i in range(DC // 2):
                            nc.tensor.matmul(
                                pv,
                                lhsT=xkt_sb[:, 2 * i:2 * i + 2,
                                            sb * P:(sb + 1) * P],
                                rhs=wv_sb[:, 2 * i:2 * i + 2,
                                          vh * 512:(vh + 1) * 512],
                                start=(i == 0), stop=False,
                                perf_mode=DR,
                            )
                        nc.tensor.matmul(
                            pv, lhsT=krows[0:2, sb * P:(sb + 1) * P],
                            rhs=csv_sb[:, vh * 512:(vh + 1) * 512],
                            start=False, stop=True)
                        nc.vector.tensor_scalar(
                            out=va_sb[:, sb, vh * 8:(vh + 1) * 8, 0:HD],
                            in0=pv.rearrange("p (h k) -> p h k", h=8),
                            scalar1=rstdoc_sb[:, sb:sb + 1], scalar2=None,
                            op0=ALU.mult)
                    nc.vector.tensor_scalar(
                        out=va_sb[:, sb, :, HD:HD + 1],
                        in0=onesh_sb.rearrange("p (h o) -> p h o", o=1),
                        scalar1=oc_sb[:, sb:sb + 1], scalar2=None,
                        op0=ALU.mult)

            den8s = [awork.tile([8, TOK], F32, tag="den8", name=f"den8_{i}")
                     for i in range(2)]

            def ctx_pair(pair):
                et = exp_bufs[pair % 3]
                dstage = awork.tile([1, 2, TOK], F32, tag="dstage",
                                    name=f"dstage_{pair}")
                for hi in range(2):
                    h = pair * 2 + hi
                    pc = cx_psum.tile([HD + 1, TOK], F32, tag="cx")
                    for tg in range(NCHP // 2):
                        nc.tensor.matmul(
                            pc,
                            lhsT=va_sb[:, 2 * tg:2 * tg + 2, h, :],
                            rhs=et[:, 2 * tg:2 * tg + 2, hi, :],
                            start=(tg == 0), stop=(tg == NCHP // 2 - 1),
                            perf_mode=DR,
                        )
                    po = 64 * hi
                    nc.vector.tensor_copy(
                        out=ctxu_sb[po:po + 64, pair, :], in_=pc[0:HD, :])
                    nc.vector.tensor_copy(
                        out=dstage[:, hi, :], in_=pc[HD:HD + 1, :])
                j = pair % 4
                nc.sync.dma_start(
                    out=den8s[pair // 4][2 * j:2 * j + 2, :], in_=dstage)

            def norm_batch(bi):
                den8 = den8s[bi]
                den8r = awork.tile([8, TOK], F32, tag="den8r")
                dscr = awork.tile([8, TOK], F32, tag="dscr")
                nc.vector.reciprocal_approx_accurate(
                    out=den8r, in_=den8, scratch=dscr)
                rc8 = awork.tile([8, TOK], BF16, tag="rc8")
                nc.vector.tensor_copy(out=rc8, in_=den8r)
                rcflat = awork.tile([1, 8, TOK], BF16, tag="rcflat")
                nc.sync.dma_start(out=rcflat, in_=rc8)
                for j in range(4):
                    pj = 4 * bi + j
                    pb = cx_psum.tile([P, TOK], F32, tag="cx")
                    for hi in range(2):
                        po = 64 * hi
                        nc.tensor.matmul(
                            pb[po:po + 64, :], lhsT=ones_sb,
                            rhs=rcflat[:, 2 * j + hi, :],
                            start=True, stop=True,
                            tile_position=(0, po))
                    nc.vector.tensor_mul(
                        out=ctxn_sb[:, pj, :],
                        in0=ctxu_sb[:, pj, :], in1=pb)

            # ---- pipelined emission: stats for q + first key span, then
            # scores of pair 0 start while remaining key stats stream in
            for sb in range(SBLK):
                stats_tile(xq_tiles[sb], qrows, sb * P)
            for sb in range(4):
                stats_tile(xk_tiles[sb], krows, sb * P, key_idx=sb)
            # rstd_q broadcast tile
            pbq = cx_psum.tile([P, TOK], F32, tag="cx")
            nc.tensor.matmul(pbq, lhsT=ones1b, rhs=qrstd_row,
                             start=True, stop=True)
            nc.vector.tensor_copy(out=rstdB, in_=pbq)

            q_pair(0)
            k_span(0, 0, 0)
            for tb in range(4):
                scores_tb(0, tb)
            for sb in range(4, 8):
                stats_tile(xk_tiles[sb], krows, sb * P, key_idx=sb)
            k_span(0, 512, 1)
            for tb in range(4, 8):
                scores_tb(0, tb)
            stats_tile(xk_tiles[8], krows, 8 * P, key_idx=8)
            k_span(0, 1024, 2)
            scores_tb(0, 8)

            for pair in range(1, DC):
                q_pair(pair)
                for s, c0 in enumerate(range(0, TKEY, 512)):
                    k_span(pair, c0, s + pair)
                for tb in range(NCH):
                    scores_tb(pair, tb)
                if pair == 1:
                    v_build(range(0, 5))
                elif pair == 2:
                    v_build(range(5, NCH))
                if pair >= 2:
                    ctx_pair(pair - 2)
                if pair == 7:
                    norm_batch(0)
            ctx_pair(DC - 2)
            ctx_pair(DC - 1)
            norm_batch(1)

        # --- Wo (fp8 DoubleRow) + rank-1 bo + residual + LN2 ---
        with tc.tile_pool(name="wo_psum", bufs=2, space="PSUM") as wo_psum, \
             tc.tile_pool(name="wo_wk", bufs=4) as wwork:
            for sb in range(SBLK):
                pw = wo_psum.tile([P, D], F32, tag="wo")
                for oh in range(2):
                    for i in range(DC // 2):
                        nc.tensor.matmul(
                            pw[:, oh * 512:(oh + 1) * 512],
                            lhsT=ctxn_sb[:, 2 * i:2 * i + 2, sb * P:(sb + 1) * P],
                            rhs=wo_sb[:, 2 * i:2 * i + 2, oh * 512:(oh + 1) * 512],
                            start=(i == 0), stop=False,
                            perf_mode=DR,
                        )
                    nc.tensor.matmul(
                        pw[:, oh * 512:(oh + 1) * 512],
                        lhsT=ones1b, rhs=bor_sb[:, oh * 512:(oh + 1) * 512],
                        start=False, stop=True)
                rs = resid_sb[:, sb, :]
                nc.vector.tensor_add(out=rs, in0=pw, in1=x_tiles[sb])
                z2 = wwork.tile([P, D], BF16, tag="z2", name=f"z2_{sb}")
                _ln_tile(nc, wwork, rs, z2, eps_sb)
                for dc in range(DC):
                    pt = wo_psum.tile([P, P], BF16, tag="tp2")
                    nc.tensor.transpose(pt, z2[:, dc * P:(dc + 1) * P], ident)
                    nc.vector.tensor_copy(
                        out=z2t_sb[:, dc, sb * P:(sb + 1) * P], in_=pt)

        # --- FFN (bf16; fp8 fails the error budget) ---
        ffn = ctx.enter_context(tc.tile_pool(name="ffn", bufs=1))
        h1t_sb = ffn.tile([P, FB, TOK], BF16)
        with tc.tile_pool(name="ff_psum", bufs=2, space="PSUM") as fa_psum, \
             tc.tile_pool(name="ffb_psum", bufs=4, space="PSUM") as fb_psum, \
             tc.tile_pool(name="ff_w1", bufs=6) as w1pool, \
             tc.tile_pool(name="ff_w2", bufs=6) as w2pool, \
             tc.tile_pool(name="ff_wk", bufs=4) as fwork:
            po0_tiles = [fb_psum.tile([P, 512], F32, tag="ffb", name=f"po0_{sb}")
                         for sb in range(SBLK)]
            for fb in range(FB):
                w1t = w1pool.tile([P, DC, P], BF16, tag="w1t")
                nc.gpsimd.dma_start(
                    out=w1t,
                    in_=w1_d[:, fb * P:(fb + 1) * P].rearrange(
                        "(c p) f -> p c f", p=P))
                pf = fa_psum.tile([P, TOK], F32, tag="ffa")
                for dc in range(DC):
                    nc.tensor.matmul(
                        pf, lhsT=w1t[:, dc, :], rhs=z2t_sb[:, dc, :],
                        start=(dc == 0), stop=(dc == DC - 1))
                nc.scalar.activation(
                    out=h1t_sb[:, fb, :], in_=pf, func=AF.Relu,
                    bias=b1_sb[:, fb:fb + 1], scale=1.0)
                w2t = w2pool.tile([P, 512], BF16, tag="w2t")
                nc.gpsimd.dma_start(out=w2t, in_=w2_d[:, fb * D:fb * D + 512])
                for sb in range(SBLK):
                    nc.tensor.matmul(
                        po0_tiles[sb], lhsT=h1t_sb[:, fb, sb * P:(sb + 1) * P],
                        rhs=w2t,
                        start=(fb == 0), stop=False)
            for sb in range(SBLK):
                nc.tensor.matmul(
                    po0_tiles[sb], lhsT=ones1b, rhs=b2r_sb[:, 0:512],
                    start=False, stop=True)
                ot = fwork.tile([P, 512], F32, tag="out")
                nc.vector.tensor_add(out=ot, in0=po0_tiles[sb],
                                     in1=resid_sb[:, sb, 0:512])
                nc.sync.dma_start(out=out_d[sb * P:(sb + 1) * P, 0:512], in_=ot)
            po1_tiles = [fb_psum.tile([P, 512], F32, tag="ffb", name=f"po1_{sb}")
                         for sb in range(SBLK)]
            for fb in range(FB):
                w2t = w2pool.tile([P, 512], BF16, tag="w2t")
                nc.gpsimd.dma_start(
                    out=w2t, in_=w2_d[:, fb * D + 512:(fb + 1) * D])
                for sb in range(SBLK):
                    nc.tensor.matmul(
                        po1_tiles[sb], lhsT=h1t_sb[:, fb, sb * P:(sb + 1) * P],
                        rhs=w2t,
                        start=(fb == 0), stop=False)
            for sb in range(SBLK):
                nc.tensor.matmul(
                    po1_tiles[sb], lhsT=ones1b, rhs=b2r_sb[:, 512:1024],
                    start=False, stop=True)
                ot = fwork.tile([P, 512], F32, tag="out")
                nc.vector.tensor_add(out=ot, in0=po1_tiles[sb],
                                     in1=resid_sb[:, sb, 512:1024])
                nc.sync.dma_start(out=out_d[sb * P:(sb + 1) * P, 512:1024], in_=ot)

    return nc


_programs = {}
LAST_EXEC_NS = {}


def _get_program(tkey):
    if ("f", tkey) not in _programs:
        f = _build_fused(tkey)
        f.finalize()
        _programs[("f", tkey)] = f
    return _programs[("f", tkey)]


def kernel(**inputs):
    inp = {k: np.asarray(v) for k, v in inputs.items()}
    x = inp["x"].astype(np.float32).reshape(B * S, D)
    mask = inp["mask"].astype(np.int32)

    # ---- host-side weight prep (layout + LN-affine folding, fp32 math) ----
    scale = np.float32(1.0 / np.sqrt(HD))
    Wq = inp["Wq"].astype(np.float32).transpose(1, 0, 2).reshape(D, D)
    Wk = inp["Wk"].astype(np.float32).transpose(1, 0, 2).reshape(D, D)
    Wv = inp["Wv"].astype(np.float32).transpose(1, 0, 2).reshape(D, D)
    g1 = inp["ln1_g"].astype(np.float32)
    b1n = inp["ln1_b"].astype(np.float32)
    g2 = inp["ln2_g"].astype(np.float32)
    b2n = inp["ln2_b"].astype(np.float32)

    def chunk_part(w8):  # [D, D] fp8 -> [P, DC*D] with row d = dc*128+p
        return np.ascontiguousarray(
            w8.reshape(DC, P, D).transpose(1, 0, 2).reshape(P, DC * D))

    wq8 = (g1[:, None] * Wq * scale).astype(fp8_np)
    wk8 = (g1[:, None] * Wk).astype(fp8_np)
    wv8 = (g1[:, None] * Wv).astype(fp8_np)
    wq_p, wk_p, wv_p = chunk_part(wq8), chunk_part(wk8), chunk_part(wv8)
    bqf = ((b1n @ Wq) * scale
           + inp["bq"].astype(np.float32).reshape(-1) * scale)
    bvf = (b1n @ Wv) + inp["bv"].astype(np.float32).reshape(-1)
    csq = np.ascontiguousarray(np.stack(
        [wq8.astype(np.float32).sum(0), bqf]).astype(bf16_np))
    csk = np.ascontiguousarray(
        wk8.astype(np.float32).sum(0)[None, :].astype(bf16_np))
    csv = np.ascontiguousarray(np.stack(
        [wv8.astype(np.float32).sum(0), bvf]).astype(bf16_np))

    wo_p = chunk_part(inp["Wo"].astype(np.float32).astype(fp8_np))
    bor = np.ascontiguousarray(
        inp["bo"].astype(np.float32)[None, :].astype(bf16_np))
    b2r = np.ascontiguousarray(
        inp["b2"].astype(np.float32)[None, :].astype(bf16_np))
    w1_p = np.ascontiguousarray(
        (g2[:, None] * inp["W1"].astype(np.float32)).astype(bf16_np))
    b1_p = np.ascontiguousarray(
        ((b2n @ inp["W1"].astype(np.float32)) + inp["b1"].astype(np.float32))
        .reshape(FB, P).T).astype(np.float32)
    w2_p = np.ascontiguousarray(
        inp["W2"].astype(np.float32).astype(bf16_np)
        .reshape(FB, P, D).transpose(1, 0, 2).reshape(P, FB * D))

    counts = [int((mask[b] == 1).sum()) for b in range(B)]
    tkey = T_PAD if max(counts) <= T_PAD else ((max(counts) + P - 1) // P) * P
    nch = tkey // P
    prog = _get_program(tkey)
    core_ids = list(range(NCORES))
    profile = bool(os.environ.get("KERNEL_PROFILE"))
    kw = {"trace": True} if profile else {}

    # fp8 token-major copy of x (stats + transposed GEMM operands)
    x8 = x.astype(fp8_np)

    # per-batch compacted fp8 key tokens (token-major + transposed) + oc
    xk_b, xkt_b, oc_b = [], [], []
    for b in range(B):
        idx = np.nonzero(mask[b] == 1)[0]
        n = len(idx)
        xk8 = np.zeros((tkey, D), fp8_np)
        xk8[:n] = x8[b * S:(b + 1) * S][idx]
        xkt = np.ascontiguousarray(
            xk8.T.reshape(DC, P, tkey).transpose(1, 0, 2).reshape(P, DC * tkey))
        oc = np.zeros((nch * P,), np.float32)
        oc[:n] = 1.0
        xk_b.append(np.ascontiguousarray(xk8))
        xkt_b.append(xkt)
        oc_b.append(np.ascontiguousarray(oc.reshape(nch, P).T))

    in_maps = []
    for c in range(NCORES):
        b = c // 4
        xq8 = np.ascontiguousarray(x8[c * TOK:(c + 1) * TOK, :])
        xt = np.ascontiguousarray(
            xq8.T.reshape(DC, P, TOK).transpose(1, 0, 2).reshape(P, DC * TOK))
        in_maps.append({
            "x": np.ascontiguousarray(x[c * TOK:(c + 1) * TOK, :]),
            "xq": xq8, "xt": xt,
            "xk": xk_b[b], "xkt": xkt_b[b], "oc": oc_b[b],
            "wq": wq_p, "wk": wk_p, "wv": wv_p,
            "csq": csq, "csk": csk, "csv": csv,
            "wo": wo_p, "bor": bor, "b2r": b2r,
            "w1": w1_p, "b1": b1_p, "w2": w2_p,
        })
    r = run_bass_kernel_spmd(prog, in_maps, core_ids, **kw)

    if profile:
        LAST_EXEC_NS.clear()
        LAST_EXEC_NS["l1"] = 0
        LAST_EXEC_NS["l2"] = r.exec_time_ns
        LAST_EXEC_NS["l2_trace"] = getattr(r, "instructions_and_trace", None)

    out = np.concatenate([r.results[c]["out"] for c in range(NCORES)], axis=0)
    return out.reshape(B, S, D).astype(np.float32)


# revision 21
# speedup vs baseline: 1.2400x; 1.2400x over previous
"""Trainium2 Bass kernel for a dense transformer decoder layer.

Shapes (hardcoded): B=2, S=2048, D=1024, H=16, HD=64, FF=4096, fp32 I/O.

Single fused launch, token-parallel over 8 cores (512 query tokens each;
batch b owned by cores 4b..4b+3). Each core redundantly computes K/V for
its batch's mask-compacted key tokens (host drops mask==0 keys and
zero-pads to a chunk multiple).

LN1 is folded into the QKV GEMMs algebraically so the preamble has no
normalize pass and no activation transposes: the host ships x^T (a pure
layout transform) in fp8 and the GEMMs run on raw x^T, with the per-token
mean handled as a rank-2 correction appended to each PSUM accumulation
chain (lhsT = [colsum(W); bias], rhs = [-mu; std] rows) and the 1/std
factor applied per-partition at the cheapest point downstream:
  - K: deferred all the way to the softmax exp (activation scale AP),
  - Q: one broadcast-tile multiply on the PSUM->SBUF copy,
  - V: folded into the existing rstd*oc per-partition copy multiply.
K's bias is dropped entirely (softmax-invariant). bo and b2 are applied
as rank-1 rows appended to the Wo / W2 PSUM chains instead of full-tile
adds. Token stats come from token-major fp8 copies via bn_stats.

Precision: fp8e4m3 with DoubleRow matmuls for QKV projections, scores
operands, exp weights, V, and Wo; LN statistics, residual stream and
softmax normalization fp32; FFN in bf16 (fp8 exceeds the error budget).
"""

import os
import numpy as np
import ml_dtypes
from contextlib import ExitStack

import concourse.bass as bass
from concourse import bacc
import concourse.mybir as mybir
import concourse.tile as tile
from concourse.bass_utils import run_bass_kernel_spmd
from concourse.masks import make_identity

B, S, D, H, FF = 2, 2048, 1024, 16, 4096
HD = D // H
EPS = 1e-5
NCORES = 8
TOK = (B * S) // NCORES          # 512 query tokens per core
P = 128
DC = D // P                      # 8 contraction chunks
SBLK = TOK // P                  # 4 s-blocks of 128
FB = FF // P                     # 32 ff blocks of 128

F32 = mybir.dt.float32
BF16 = mybir.dt.bfloat16
FP8 = mybir.dt.float8e4
AF = mybir.ActivationFunctionType
ALU = mybir.AluOpType
DR = mybir.MatmulPerfMode.DoubleRow

bf16_np = ml_dtypes.bfloat16
fp8_np = ml_dtypes.float8_e4m3

T_PAD = 1152  # compacted key columns (only mask==1 keys kept, zero-padded)


def _ln_tile(nc, pools, x_tile, out_tile, eps_sb):
    """LayerNorm (no affine) of one [128, D] fp32 tile into out_tile."""
    stats = pools.tile([P, 2, 6], F32, tag="ln_stats")
    mv = pools.tile([P, 2], F32, tag="ln_mv")
    xg = x_tile.rearrange("p (g d) -> p g d", g=2)
    for g in range(2):
        nc.vector.bn_stats(out=stats[:, g, :], in_=xg[:, g, :])
    nc.vector.bn_aggr(out=mv[:], in_=stats[:])
    mean = mv[:, 0:1]
    std = pools.tile([P, 1], F32, tag="ln_std")
    nc.scalar.activation(out=std, in_=mv[:, 1:2], func=AF.Sqrt, bias=eps_sb, scale=1.0)
    nc.vector.reciprocal(out=std, in_=std)
    nc.vector.tensor_scalar(
        out=out_tile,
        in0=x_tile,
        scalar1=mean,
        scalar2=std,
        op0=ALU.subtract,
        op1=ALU.mult,
    )


def _build_fused(TKEY=T_PAD):
    NCH = TKEY // P                  # real key chunks (9 for 1152)
    NCHP = NCH + (NCH % 2)           # padded to even for DoubleRow (10)
    NSP = TKEY // 384                # key spans (3 for 1152)
    nc = bacc.Bacc(None, target_bir_lowering=False, debug=False)
    x_d = nc.declare_dram_parameter("x", [TOK, D], F32, isOutput=False)
    xk_d = nc.declare_dram_parameter("xk", [TKEY, D], FP8, isOutput=False)
    xt_d = nc.declare_dram_parameter("xt", [P, DC * TOK], FP8, isOutput=False)
    xkt_d = nc.declare_dram_parameter("xkt", [P, NSP * DC * 384], FP8,
                                      isOutput=False)
    oc_d = nc.declare_dram_parameter("oc", [P, NCH], F32, isOutput=False)
    wq_d = nc.declare_dram_parameter("wq", [P, DC * D], FP8, isOutput=False)
    wk_d = nc.declare_dram_parameter("wk", [P, DC * D], FP8, isOutput=False)
    wv_d = nc.declare_dram_parameter("wv", [P, DC * D], FP8, isOutput=False)
    csq_d = nc.declare_dram_parameter("csq", [2, D], BF16, isOutput=False)
    csk_d = nc.declare_dram_parameter("csk", [1, D], BF16, isOutput=False)
    csv_d = nc.declare_dram_parameter("csv", [2, D], BF16, isOutput=False)
    wo_d = nc.declare_dram_parameter("wo", [P, DC * D], FP8, isOutput=False)
    bor_d = nc.declare_dram_parameter("bor", [1, D], BF16, isOutput=False)
    b2r_d = nc.declare_dram_parameter("b2r", [1, D], BF16, isOutput=False)
    w1_d = nc.declare_dram_parameter("w1", [D, FF], BF16, isOutput=False)
    b1_d = nc.declare_dram_parameter("b1", [P, FB], F32, isOutput=False)
    w2_d = nc.declare_dram_parameter("w2", [P, FB * D], BF16, isOutput=False)
    out_d = nc.declare_dram_parameter("out", [TOK, D], F32, isOutput=True)

    with tile.TileContext(nc) as tc, ExitStack() as ctx:
        glob = ctx.enter_context(tc.tile_pool(name="glob", bufs=1))

        ident = glob.tile([P, P], BF16)
        make_identity(nc, ident)
        eps_sb = glob.tile([P, 1], F32)
        nc.vector.memset(eps_sb, EPS)
        ones_sb = glob.tile([1, 64], BF16)
        nc.vector.memset(ones_sb, 1.0)
        ones1b = glob.tile([1, P], BF16)
        nc.vector.memset(ones1b, 1.0)
        negone_sb = glob.tile([P, 1], F32)
        nc.vector.memset(negone_sb, -1.0)
        onesh_sb = glob.tile([P, H], F32)
        nc.vector.memset(onesh_sb, 1.0)

        qt_sb = glob.tile([P, DC, TOK], FP8)
        kt_sb = glob.tile([P, DC, TKEY], FP8)
        va_sb = glob.tile([P, NCHP, H, HD + 1], FP8)
        ctxu_sb = glob.tile([P, DC, TOK], BF16)      # unnormalized ctx^T
        ctxn_sb = glob.tile([P, DC, TOK], FP8)       # normalized ctx^T
        exp_bufs = [glob.tile([P, NCHP, 2, TOK], FP8, name=f"expb{i}")
                    for i in range(3)]
        if NCHP != NCH:
            for eb in exp_bufs:
                nc.gpsimd.memset(eb[:, NCH:NCHP, :, :], 0.0)
            nc.gpsimd.memset(va_sb[:, NCH:NCHP, :, :], 0.0)

        # ---- input DMAs; stats-feeding tiles go first on the sync queue
        oc_sb = glob.tile([P, NCH], F32)
        nc.sync.dma_start(out=oc_sb, in_=oc_d[:, :])
        bor_sb = glob.tile([1, D], BF16)
        b2r_sb = glob.tile([1, D], BF16)
        b1_sb = glob.tile([P, FB], F32)
        wo_sb = glob.tile([P, DC, D], FP8)
        w1t0 = glob.tile([P, DC, P], BF16)       # FFN fb=0 prefetch
        w2t0 = glob.tile([P, 512], BF16)
        resid_sb = glob.tile([P, SBLK, D], F32)
        z2t_sb = glob.tile([P, DC, TOK], BF16)
        x_tiles = []

        with tc.tile_pool(name="qkv", bufs=1) as qkvp, \
             tc.tile_pool(name="lnw", bufs=2) as lnw, \
             tc.tile_pool(name="attn_sc", bufs=3, space="PSUM") as sc_psum, \
             tc.tile_pool(name="attn_cx", bufs=2, space="PSUM") as cx_psum, \
             tc.tile_pool(name="attn_wk", bufs=1) as awork:
            for sb in range(SBLK):
                xt_ = glob.tile([P, D], F32, name=f"x{sb}")
                nc.sync.dma_start(out=xt_, in_=x_d[sb * P:(sb + 1) * P, :])
                x_tiles.append(xt_)
            xk_tiles = []
            for sb in range(NCH):
                t = qkvp.tile([P, D], FP8, name=f"xk{sb}")
                nc.sync.dma_start(out=t, in_=xk_d[sb * P:(sb + 1) * P, :])
                xk_tiles.append(t)
            xt_sb = qkvp.tile([P, DC, TOK], FP8)
            nc.sync.dma_start(
                out=xt_sb, in_=xt_d[:].rearrange("p (c n) -> p c n", c=DC))
            xkt_sb = qkvp.tile([P, NSP, DC, 384], FP8)
            xkt_ap = xkt_d[:].rearrange("p (s c n) -> p s c n", s=NSP, c=DC)
            for s in range(NSP):
                nc.sync.dma_start(out=xkt_sb[:, s, :, :],
                                  in_=xkt_ap[:, s, :, :])

            # weights on the gpsimd issue queue (wo/biases deferred to
            # mid-attention so the preamble DMA window stays clear)
            wq_sb = qkvp.tile([P, DC, D], FP8)
            wk_sb = qkvp.tile([P, DC, D], FP8)
            wv_sb = qkvp.tile([P, DC, D], FP8)
            for dc in range(DC):
                nc.gpsimd.dma_start(out=wk_sb[:, dc, :],
                                    in_=wk_d[:, dc * D:(dc + 1) * D])
                nc.gpsimd.dma_start(out=wq_sb[:, dc, :],
                                    in_=wq_d[:, dc * D:(dc + 1) * D])
            csq_sb = qkvp.tile([2, D], BF16)
            csk_sb = qkvp.tile([1, D], BF16)
            csv_sb = qkvp.tile([2, D], BF16)
            nc.gpsimd.dma_start(out=csk_sb, in_=csk_d[:, :])
            nc.gpsimd.dma_start(out=csq_sb, in_=csq_d[:, :])
            nc.gpsimd.dma_start(out=csv_sb, in_=csv_d[:, :])
            for dc in range(DC):
                nc.gpsimd.dma_start(out=wv_sb[:, dc, :],
                                    in_=wv_d[:, dc * D:(dc + 1) * D])

            # ---- per-token LN1 stats (no normalize, no transposed z1)
            NT = SBLK + NCH                      # 13 stat tiles (q then k)
            qrows = qkvp.tile([3, TOK], BF16)    # rows: -mu, std, rstd
            qrstd_row = qkvp.tile([1, TOK], BF16)
            krows = qkvp.tile([3, TKEY], BF16)
            mvall = qkvp.tile([P, NT, 2], F32)   # bn_aggr mean/var columns
            stds_sb = qkvp.tile([P, NT], F32)
            rstds_sb = qkvp.tile([P, NT], F32)   # cols 4.. = key rstd (exp)
            sall = qkvp.tile([P, NT, 3], BF16)   # (-mu, std, rstd) bf16
            rstdoc_sb = qkvp.tile([P, NCH], F32)
            rstdB = qkvp.tile([P, TOK], BF16)    # broadcast rstd_q

            def stats_bn(src, i):
                bstats = lnw.tile([P, 2, 6], F32, tag="bn")
                xg = src.rearrange("p (g d) -> p g d", g=2)
                nc.vector.bn_stats(out=bstats[:, 0, :], in_=xg[:, 0, :])
                nc.vector.bn_stats(out=bstats[:, 1, :], in_=xg[:, 1, :])
                nc.vector.bn_aggr(out=mvall[:, i, :], in_=bstats[:])

            def stats_finish():
                # one batched sqrt/recip/negate/cast pass for all tiles
                nc.scalar.activation(out=stds_sb, in_=mvall[:, :, 1],
                                     func=AF.Sqrt, bias=eps_sb, scale=1.0)
                nc.vector.reciprocal(out=rstds_sb, in_=stds_sb)
                nc.vector.tensor_scalar(out=sall[:, :, 0],
                                        in0=mvall[:, :, 0],
                                        scalar1=negone_sb, scalar2=None,
                                        op0=ALU.mult)
                nc.vector.tensor_copy(out=sall[:, :, 1], in_=stds_sb)
                nc.vector.tensor_copy(out=sall[:, :, 2], in_=rstds_sb)
                nc.vector.tensor_mul(out=rstdoc_sb, in0=rstds_sb[:, SBLK:],
                                     in1=oc_sb)
                for i in range(NT):
                    pt = sc_psum.tile([3, P], BF16, tag="sc")
                    nc.tensor.transpose(pt, sall[:, i, :], ident)
                    if i < SBLK:
                        nc.vector.tensor_copy(
                            out=qrows[:, i * P:(i + 1) * P], in_=pt)
                        pt1 = sc_psum.tile([1, P], BF16, tag="sc")
                        nc.tensor.transpose(pt1, sall[:, i, 2:3], ident)
                        nc.vector.tensor_copy(
                            out=qrstd_row[:, i * P:(i + 1) * P], in_=pt1)
                    else:
                        k = i - SBLK
                        nc.vector.tensor_copy(
                            out=krows[:, k * P:(k + 1) * P], in_=pt)

            def q_pair(pair):
                pq = cx_psum.tile([P, 512], F32, tag="cx")
                for i in range(DC // 2):
                    nc.tensor.matmul(
                        pq,
                        lhsT=wq_sb[:, 2 * i:2 * i + 2, pair * P:(pair + 1) * P],
                        rhs=xt_sb[:, 2 * i:2 * i + 2, :],
                        start=(i == 0), stop=False,
                        perf_mode=DR,
                    )
                nc.tensor.matmul(
                    pq, lhsT=csq_sb[:, pair * P:(pair + 1) * P],
                    rhs=qrows[0:2, :], start=False, stop=True)
                nc.vector.tensor_mul(out=qt_sb[:, pair, :], in0=pq, in1=rstdB)

            def k_span(pair, s, eng):
                c0 = s * 384
                pk = cx_psum.tile([P, 384], F32, tag="cx")
                for i in range(DC // 2):
                    nc.tensor.matmul(
                        pk,
                        lhsT=wk_sb[:, 2 * i:2 * i + 2, pair * P:(pair + 1) * P],
                        rhs=xkt_sb[:, s, 2 * i:2 * i + 2, :],
                        start=(i == 0), stop=False,
                        perf_mode=DR,
                    )
                nc.tensor.matmul(
                    pk, lhsT=csk_sb[:, pair * P:(pair + 1) * P],
                    rhs=krows[0:1, c0:c0 + 384], start=False, stop=True)
                eng.tensor_copy(out=kt_sb[:, pair, c0:c0 + 384], in_=pk)

            def scores_tb(pair, tb):
                et = exp_bufs[pair % 3]
                ps = sc_psum.tile([P, 2, TOK], F32, tag="sc")
                for hi in range(2):
                    po = 64 * hi
                    nc.tensor.matmul(
                        ps[:, hi, :],
                        lhsT=kt_sb[po:po + 64, pair, tb * P:(tb + 1) * P],
                        rhs=qt_sb[po:po + 64, pair, :],
                        start=True, stop=True,
                        tile_position=(po, 0),
                    )
                nc.scalar.activation(
                    out=et[:, tb, :, :], in_=ps, func=AF.Exp,
                    bias=negone_sb,
                    scale=rstds_sb[:, SBLK + tb:SBLK + tb + 1])

            def v_build(sbs):
                for sb in sbs:
                    off = (sb % 3) * P
                    for vh in range(2):
                        pv = cx_psum.tile([P, 512], F32, tag="cx")
                        for i in range(DC // 2):
                            nc.tensor.matmul(
                                pv,
                                lhsT=xkt_sb[:, sb // 3, 2 * i:2 * i + 2,
                                            off:off + P],
                                rhs=wv_sb[:, 2 * i:2 * i + 2,
                                          vh * 512:(vh + 1) * 512],
                                start=(i == 0), stop=False,
                                perf_mode=DR,
                            )
                        nc.tensor.matmul(
                            pv, lhsT=krows[0:2, sb * P:(sb + 1) * P],
                            rhs=csv_sb[:, vh * 512:(vh + 1) * 512],
                            start=False, stop=True)
                        nc.vector.tensor_scalar(
                            out=va_sb[:, sb, vh * 8:(vh + 1) * 8, 0:HD],
                            in0=pv.rearrange("p (h k) -> p h k", h=8),
                            scalar1=rstdoc_sb[:, sb:sb + 1], scalar2=None,
                            op0=ALU.mult)
                    nc.vector.tensor_scalar(
                        out=va_sb[:, sb, :, HD:HD + 1],
                        in0=onesh_sb.rearrange("p (h o) -> p h o", o=1),
                        scalar1=oc_sb[:, sb:sb + 1], scalar2=None,
                        op0=ALU.mult)

            den8s = [awork.tile([8, TOK], F32, tag="den8", name=f"den8_{i}")
                     for i in range(2)]

            def ctx_pair(pair):
                et = exp_bufs[pair % 3]
                dstage = awork.tile([1, 2, TOK], F32, tag="dstage",
                                    name=f"dstage_{pair}")
                for hi in range(2):
                    h = pair * 2 + hi
                    pc = cx_psum.tile([HD + 1, TOK], F32, tag="cx")
                    for tg in range(NCHP // 2):
                        nc.tensor.matmul(
                            pc,
                            lhsT=va_sb[:, 2 * tg:2 * tg + 2, h, :],
                            rhs=et[:, 2 * tg:2 * tg + 2, hi, :],
                            start=(tg == 0), stop=(tg == NCHP // 2 - 1),
                            perf_mode=DR,
                        )
                    po = 64 * hi
                    nc.vector.tensor_copy(
                        out=ctxu_sb[po:po + 64, pair, :], in_=pc[0:HD, :])
                    nc.vector.tensor_copy(
                        out=dstage[:, hi, :], in_=pc[HD:HD + 1, :])
                j = pair % 4
                nc.sync.dma_start(
                    out=den8s[pair // 4][2 * j:2 * j + 2, :], in_=dstage)

            def norm_batch(bi):
                den8 = den8s[bi]
                den8r = awork.tile([8, TOK], F32, tag="den8r")
                dscr = awork.tile([8, TOK], F32, tag="dscr")
                nc.vector.reciprocal_approx_accurate(
                    out=den8r, in_=den8, scratch=dscr)
                rc8 = awork.tile([8, TOK], BF16, tag="rc8")
                nc.vector.tensor_copy(out=rc8, in_=den8r)
                rcflat = awork.tile([1, 8, TOK], BF16, tag="rcflat")
                nc.sync.dma_start(out=rcflat, in_=rc8)
                for j in range(4):
                    pj = 4 * bi + j
                    pb = cx_psum.tile([P, TOK], F32, tag="cx")
                    for hi in range(2):
                        po = 64 * hi
                        nc.tensor.matmul(
                            pb[po:po + 64, :], lhsT=ones_sb,
                            rhs=rcflat[:, 2 * j + hi, :],
                            start=True, stop=True,
                            tile_position=(0, po))
                    nc.vector.tensor_mul(
                        out=ctxn_sb[:, pj, :],
                        in0=ctxu_sb[:, pj, :], in1=pb)

            # ---- pipelined emission: all bn stats (as DMAs land), one
            # batched finish, then the pair pipeline with early exp
            for sb in range(SBLK):
                stats_bn(x_tiles[sb], sb)
            for sb in range(NCH):
                stats_bn(xk_tiles[sb], SBLK + sb)
            stats_finish()
            # rstd_q broadcast tile
            pbq = cx_psum.tile([P, TOK], F32, tag="cx")
            nc.tensor.matmul(pbq, lhsT=ones1b, rhs=qrstd_row,
                             start=True, stop=True)
            nc.vector.tensor_copy(out=rstdB, in_=pbq)

            for pair in range(DC):
                q_pair(pair)
                for s in range(NSP):
                    k_span(pair, s, nc.vector)
                    for tb in range(3 * s, 3 * s + 3):
                        scores_tb(pair, tb)
                if pair == 1:
                    v_build(range(0, 5))
                elif pair == 2:
                    v_build(range(5, NCH))
                elif pair == 3:
                    # deferred weight loads (DMA window is clear now)
                    nc.gpsimd.dma_start(
                        out=wo_sb,
                        in_=wo_d[:].rearrange("p (c n) -> p c n", c=DC))
                    nc.gpsimd.dma_start(out=bor_sb, in_=bor_d[:, :])
                    nc.gpsimd.dma_start(out=b2r_sb, in_=b2r_d[:, :])
                    nc.gpsimd.dma_start(out=b1_sb, in_=b1_d[:, :])
                    nc.gpsimd.dma_start(
                        out=w1t0,
                        in_=w1_d[:, 0:P].rearrange("(c p) f -> p c f", p=P))
                    nc.gpsimd.dma_start(out=w2t0, in_=w2_d[:, 0:512])
                if pair >= 2:
                    ctx_pair(pair - 2)
                if pair == 7:
                    norm_batch(0)
            ctx_pair(DC - 2)
            ctx_pair(DC - 1)
            norm_batch(1)

        # --- Wo (fp8 DoubleRow) + rank-1 bo + residual + LN2 ---
        with tc.tile_pool(name="wo_psum", bufs=2, space="PSUM") as wo_psum, \
             tc.tile_pool(name="wo_wk", bufs=4) as wwork:
            for sb in range(SBLK):
                pw = wo_psum.tile([P, D], F32, tag="wo")
                for oh in range(2):
                    for i in range(DC // 2):
                        nc.tensor.matmul(
                            pw[:, oh * 512:(oh + 1) * 512],
                            lhsT=ctxn_sb[:, 2 * i:2 * i + 2, sb * P:(sb + 1) * P],
                            rhs=wo_sb[:, 2 * i:2 * i + 2, oh * 512:(oh + 1) * 512],
                            start=(i == 0), stop=False,
                            perf_mode=DR,
                        )
                    nc.tensor.matmul(
                        pw[:, oh * 512:(oh + 1) * 512],
                        lhsT=ones1b, rhs=bor_sb[:, oh * 512:(oh + 1) * 512],
                        start=False, stop=True)
                rs = resid_sb[:, sb, :]
                nc.vector.tensor_add(out=rs, in0=pw, in1=x_tiles[sb])
                z2 = wwork.tile([P, D], BF16, tag="z2", name=f"z2_{sb}")
                _ln_tile(nc, wwork, rs, z2, eps_sb)
                for dc in range(DC):
                    pt = wo_psum.tile([P, P], BF16, tag="tp2")
                    nc.tensor.transpose(pt, z2[:, dc * P:(dc + 1) * P], ident)
                    if dc % 2 == 0:
                        nc.vector.tensor_copy(
                            out=z2t_sb[:, dc, sb * P:(sb + 1) * P], in_=pt)
                    else:
                        nc.scalar.copy(
                            out=z2t_sb[:, dc, sb * P:(sb + 1) * P], in_=pt)

        # --- FFN (bf16; fp8 fails the error budget) ---
        ffn = ctx.enter_context(tc.tile_pool(name="ffn", bufs=1))
        h1t_sb = ffn.tile([P, FB, TOK], BF16)
        with tc.tile_pool(name="ff_psum", bufs=2, space="PSUM") as fa_psum, \
             tc.tile_pool(name="ffb_psum", bufs=4, space="PSUM") as fb_psum, \
             tc.tile_pool(name="ff_w1", bufs=6) as w1pool, \
             tc.tile_pool(name="ff_w2", bufs=6) as w2pool, \
             tc.tile_pool(name="ff_wk", bufs=4) as fwork:
            po0_tiles = [fb_psum.tile([P, 512], F32, tag="ffb", name=f"po0_{sb}")
                         for sb in range(SBLK)]
            for fb in range(FB):
                if fb == 0:
                    w1t = w1t0
                else:
                    w1t = w1pool.tile([P, DC, P], BF16, tag="w1t")
                    nc.gpsimd.dma_start(
                        out=w1t,
                        in_=w1_d[:, fb * P:(fb + 1) * P].rearrange(
                            "(c p) f -> p c f", p=P))
                pf = fa_psum.tile([P, TOK], F32, tag="ffa")
                for dc in range(DC):
                    nc.tensor.matmul(
                        pf, lhsT=w1t[:, dc, :], rhs=z2t_sb[:, dc, :],
                        start=(dc == 0), stop=(dc == DC - 1))
                nc.scalar.activation(
                    out=h1t_sb[:, fb, :], in_=pf, func=AF.Relu,
                    bias=b1_sb[:, fb:fb + 1], scale=1.0)
                if fb == 0:
                    w2t = w2t0
                else:
                    w2t = w2pool.tile([P, 512], BF16, tag="w2t")
                    nc.gpsimd.dma_start(out=w2t,
                                        in_=w2_d[:, fb * D:fb * D + 512])
                for sb in range(SBLK):
                    nc.tensor.matmul(
                        po0_tiles[sb], lhsT=h1t_sb[:, fb, sb * P:(sb + 1) * P],
                        rhs=w2t,
                        start=(fb == 0), stop=False)
            for sb in range(SBLK):
                nc.tensor.matmul(
                    po0_tiles[sb], lhsT=ones1b, rhs=b2r_sb[:, 0:512],
                    start=False, stop=True)
                ot = fwork.tile([P, 512], F32, tag="out")
                nc.vector.tensor_add(out=ot, in0=po0_tiles[sb],
                                     in1=resid_sb[:, sb, 0:512])
                nc.sync.dma_start(out=out_d[sb * P:(sb + 1) * P, 0:512], in_=ot)
            po1_tiles = [fb_psum.tile([P, 512], F32, tag="ffb", name=f"po1_{sb}")
                         for sb in range(SBLK)]
            for fb in range(FB):
                w2t = w2pool.tile([P, 512], BF16, tag="w2t")
                nc.gpsimd.dma_start(
                    out=w2t, in_=w2_d[:, fb * D + 512:(fb + 1) * D])
                for sb in range(SBLK):
                    nc.tensor.matmul(
                        po1_tiles[sb], lhsT=h1t_sb[:, fb, sb * P:(sb + 1) * P],
                        rhs=w2t,
                        start=(fb == 0), stop=False)
            for sb in range(SBLK):
                nc.tensor.matmul(
                    po1_tiles[sb], lhsT=ones1b, rhs=b2r_sb[:, 512:1024],
                    start=False, stop=True)
                ot = fwork.tile([P, 512], F32, tag="out")
                nc.vector.tensor_add(out=ot, in0=po1_tiles[sb],
                                     in1=resid_sb[:, sb, 512:1024])
                nc.sync.dma_start(out=out_d[sb * P:(sb + 1) * P, 512:1024], in_=ot)

    return nc


_programs = {}
LAST_EXEC_NS = {}


def _get_program(tkey):
    if ("f", tkey) not in _programs:
        f = _build_fused(tkey)
        f.finalize()
        _programs[("f", tkey)] = f
    return _programs[("f", tkey)]


def kernel(**inputs):
    inp = {k: np.asarray(v) for k, v in inputs.items()}
    x = inp["x"].astype(np.float32).reshape(B * S, D)
    mask = inp["mask"].astype(np.int32)

    # ---- host-side weight prep (layout + LN-affine folding, fp32 math) ----
    scale = np.float32(1.0 / np.sqrt(HD))
    Wq = inp["Wq"].astype(np.float32).transpose(1, 0, 2).reshape(D, D)
    Wk = inp["Wk"].astype(np.float32).transpose(1, 0, 2).reshape(D, D)
    Wv = inp["Wv"].astype(np.float32).transpose(1, 0, 2).reshape(D, D)
    g1 = inp["ln1_g"].astype(np.float32)
    b1n = inp["ln1_b"].astype(np.float32)
    g2 = inp["ln2_g"].astype(np.float32)
    b2n = inp["ln2_b"].astype(np.float32)

    def chunk_part(w8):  # [D, D] fp8 -> [P, DC*D] with row d = dc*128+p
        return np.ascontiguousarray(
            w8.reshape(DC, P, D).transpose(1, 0, 2).reshape(P, DC * D))

    wq8 = (g1[:, None] * Wq * scale).astype(fp8_np)
    wk8 = (g1[:, None] * Wk).astype(fp8_np)
    wv8 = (g1[:, None] * Wv).astype(fp8_np)
    wq_p, wk_p, wv_p = chunk_part(wq8), chunk_part(wk8), chunk_part(wv8)
    bqf = ((b1n @ Wq) * scale
           + inp["bq"].astype(np.float32).reshape(-1) * scale)
    bvf = (b1n @ Wv) + inp["bv"].astype(np.float32).reshape(-1)
    csq = np.ascontiguousarray(np.stack(
        [wq8.astype(np.float32).sum(0), bqf]).astype(bf16_np))
    csk = np.ascontiguousarray(
        wk8.astype(np.float32).sum(0)[None, :].astype(bf16_np))
    csv = np.ascontiguousarray(np.stack(
        [wv8.astype(np.float32).sum(0), bvf]).astype(bf16_np))

    wo_p = chunk_part(inp["Wo"].astype(np.float32).astype(fp8_np))
    bor = np.ascontiguousarray(
        inp["bo"].astype(np.float32)[None, :].astype(bf16_np))
    b2r = np.ascontiguousarray(
        inp["b2"].astype(np.float32)[None, :].astype(bf16_np))
    w1_p = np.ascontiguousarray(
        (g2[:, None] * inp["W1"].astype(np.float32)).astype(bf16_np))
    b1_p = np.ascontiguousarray(
        ((b2n @ inp["W1"].astype(np.float32)) + inp["b1"].astype(np.float32))
        .reshape(FB, P).T).astype(np.float32)
    w2_p = np.ascontiguousarray(
        inp["W2"].astype(np.float32).astype(bf16_np)
        .reshape(FB, P, D).transpose(1, 0, 2).reshape(P, FB * D))

    counts = [int((mask[b] == 1).sum()) for b in range(B)]
    tkey = T_PAD if max(counts) <= T_PAD else ((max(counts) + 383) // 384) * 384
    nch = tkey // P
    prog = _get_program(tkey)
    core_ids = list(range(NCORES))
    profile = bool(os.environ.get("KERNEL_PROFILE"))
    kw = {"trace": True} if profile else {}

    # fp8 token-major copy of x (stats + transposed GEMM operands)
    x8 = x.astype(fp8_np)
    nsp = tkey // 384

    # per-batch compacted fp8 key tokens (token-major + span-major^T) + oc
    xk_b, xkt_b, oc_b = [], [], []
    for b in range(B):
        idx = np.nonzero(mask[b] == 1)[0]
        n = len(idx)
        xk8 = np.zeros((tkey, D), fp8_np)
        xk8[:n] = x8[b * S:(b + 1) * S][idx]
        xkt = np.ascontiguousarray(
            xk8.T.reshape(DC, P, nsp, 384).transpose(1, 2, 0, 3)
            .reshape(P, nsp * DC * 384))
        oc = np.zeros((nch * P,), np.float32)
        oc[:n] = 1.0
        xk_b.append(np.ascontiguousarray(xk8))
        xkt_b.append(xkt)
        oc_b.append(np.ascontiguousarray(oc.reshape(nch, P).T))

    in_maps = []
    for c in range(NCORES):
        b = c // 4
        xq8 = x8[c * TOK:(c + 1) * TOK, :]
        xt = np.ascontiguousarray(
            xq8.T.reshape(DC, P, TOK).transpose(1, 0, 2).reshape(P, DC * TOK))
        in_maps.append({
            "x": np.ascontiguousarray(x[c * TOK:(c + 1) * TOK, :]),
            "xt": xt,
            "xk": xk_b[b], "xkt": xkt_b[b], "oc": oc_b[b],
            "wq": wq_p, "wk": wk_p, "wv": wv_p,
            "csq": csq, "csk": csk, "csv": csv,
            "wo": wo_p, "bor": bor, "b2r": b2r,
            "w1": w1_p, "b1": b1_p, "w2": w2_p,
        })
    r = run_bass_kernel_spmd(prog, in_maps, core_ids, **kw)

    if profile:
        LAST_EXEC_NS.clear()
        LAST_EXEC_NS["l1"] = 0
        LAST_EXEC_NS["l2"] = r.exec_time_ns
        LAST_EXEC_NS["l2_trace"] = getattr(r, "instructions_and_trace", None)

    out = np.concatenate([r.results[c]["out"] for c in range(NCORES)], axis=0)
    return out.reshape(B, S, D).astype(np.float32)
